# revision 19
# baseline (speedup 1.0000x reference)
"""Causal self-attention Bass/Tile kernel for Trainium2, 8 NeuronCores SPMD.

Problem: B=4, T=2048, C=1024, H=16 heads, D=64, f32 in/out.
    qkv = x @ w_qkv.T; per-head causal softmax(q k^T / sqrt(D)) @ v;
    out = attn @ w_out.T + b_out.

Sharding (hybrid batch x tensor-parallel): core c handles batch b = c//2 and
head group hg = c%2 (8 of 16 heads). Each core computes a full [T, C] partial
of the output projection restricted to its heads; the host sums the two
partials per batch and adds the bias.

Per-core algorithm, SUPER-MAJOR order (i-supers of 512 query columns):
  for S in 0..3:  (with QKV-projection tiles threaded in as PE filler)
    for h in 0..7:
      for lb in 0..4S+3: scores (kT q, keys on partitions) -> exp -> PV
        - causal diag mask folded into the scores PSUM accumulation as one
          extra 128-col matmul adding -1e6 above the diagonal (ident^T @ M),
          so exp gives exact zeros and no vector-engine hop is needed.
        - PV accumulates [1|v]^T @ ex into a 1-bank [65, 512] PSUM tile;
          row 0 is the softmax denominator (finalized per super by causality).
      normalize (h, S): copy PSUM->SBUF, reciprocal_approx_fast on row 0,
        partition-broadcast the reciprocal row via a K=1 ones matmul into
        PSUM, one fused multiply writes normalized attnT bf16.
    output projection for the 4 t-blocks of S runs during super S+1.

All PSUM tiles are one bank: main pool (scores / qkv pairs / out-proj) x4,
PV accumulators x2, broadcast x2 = 8 banks.
"""

import sys

if "/opt/trn_rl_repo" not in sys.path:
    sys.path.insert(0, "/opt/trn_rl_repo")

import numpy as np
import ml_dtypes

import concourse.bass as bass
import concourse.tile as tile
import concourse.mybir as mybir
from concourse import bacc
from concourse.bass_utils import run_bass_kernel_spmd

BF16 = mybir.dt.bfloat16
F32 = mybir.dt.float32
NPBF16 = ml_dtypes.bfloat16
EXPF = mybir.ActivationFunctionType.Exp

P = 128
C = 1024
CC = C // P      # 8 contraction chunks
NH = 8           # heads per core
D = 64
J = NH * D       # 512 (local q/k/v width)
JC = J // P      # 4 j-chunks


def build_program(T=2048):
    LC = T // P          # l/t 128-blocks (16)
    NS = T // 512        # 512-wide i-supers (4)
    SCALE = 0.125        # 1/sqrt(D)

    nc = bacc.Bacc("TRN2", target_bir_lowering=False, debug=False, num_devices=8)

    xT_d = nc.dram_tensor("xT", [CC, P, T], BF16, kind="ExternalInput")
    wqkvT_d = nc.dram_tensor("wqkvT", [CC, P, 3 * J], BF16, kind="ExternalInput")
    woutT_d = nc.dram_tensor("woutT", [JC, P, C], BF16, kind="ExternalInput")
    ident_d = nc.dram_tensor("ident", [P, P], BF16, kind="ExternalInput")
    maskm_d = nc.dram_tensor("maskm", [P, P], BF16, kind="ExternalInput")
    y_d = nc.dram_tensor("y", [LC, P, C], F32, kind="ExternalOutput")

    with tile.TileContext(nc) as tc:
        with (
            tc.tile_pool(name="persist", bufs=1) as persist,
            tc.tile_pool(name="io", bufs=1) as io_pool,
            tc.tile_pool(name="dst", bufs=3) as dst_pool,
            tc.tile_pool(name="dnp", bufs=2) as dn_pool,
            tc.tile_pool(name="rec", bufs=2) as rec_pool,
            tc.tile_pool(name="recb", bufs=2) as recb_pool,
            tc.tile_pool(name="expp", bufs=4) as exp_pool,
            tc.tile_pool(name="outp", bufs=3) as out_pool,
            tc.tile_pool(name="ps_m", bufs=4, space="PSUM") as ps_m,
            tc.tile_pool(name="ps_pv", bufs=2, space="PSUM") as ps_pv,
            tc.tile_pool(name="ps_bc", bufs=2, space="PSUM") as ps_bc,
        ):
            woutT = persist.tile([P, JC, C], BF16)
            ident = persist.tile([P, P], BF16)
            maskm = persist.tile([P, P], BF16)
            ones_t = persist.tile([1, D], BF16)
            qkT = persist.tile([P, 2 * JC, T], BF16)
            v_aug = persist.tile([P, LC, NH, D + 1], BF16)
            attnT = persist.tile([P, JC, T], BF16)
            xT = io_pool.tile([P, CC, T], BF16)
            wqkvT = io_pool.tile([P, CC, 3 * J], BF16)

            # inputs: tiny mask/ident first, x/w chunks interleaved, wout last
            nc.sync.dma_start(ident[:], ident_d[:])
            nc.sync.dma_start(maskm[:], maskm_d[:])
            # chunk 0 split so the first qk matmuls start sooner
            nc.sync.dma_start(wqkvT[:, 0, :], wqkvT_d[0])
            nc.sync.dma_start(xT[:, 0, 0 : T // 2], xT_d[0][:, 0 : T // 2])
            nc.sync.dma_start(xT[:, 0, T // 2 :], xT_d[0][:, T // 2 :])
            for cc in range(1, CC):
                nc.sync.dma_start(xT[:, cc, :], xT_d[cc])
                nc.sync.dma_start(wqkvT[:, cc, :], wqkvT_d[cc])
            for jc in range(JC):
                nc.sync.dma_start(woutT[:, jc, :], woutT_d[jc])
            nc.gpsimd.memset(ones_t[:], 1.0)
            nc.gpsimd.memset(v_aug[:, :, :, D], 1.0)

            # ---------------- QKV projection pieces ----------------
            def emit_qk_tile(jc, ts, pool=None):
                """q/k chunk jc (0..7: q then k), t-super ts: [P, 512]."""
                pq = (pool or ps_m).tile([P, 512], F32, tag="m", name=f"qk{jc}_{ts}")
                for cc in range(CC):
                    nc.tensor.matmul(
                        pq[:],
                        wqkvT[:, cc, jc * P : (jc + 1) * P],
                        xT[:, cc, ts * 512 : (ts + 1) * 512],
                        start=(cc == 0),
                        stop=(cc == CC - 1),
                    )
                nc.vector.tensor_copy(
                    qkT[:, jc, ts * 512 : (ts + 1) * 512], pq[:]
                )

            def emit_v_tile(lc, pool=None, tag="m"):
                """v for t-block lc into v_aug."""
                pq = (pool or ps_m).tile([P, 512], F32, tag=tag, name=f"v{lc}")
                for cc in range(CC):
                    nc.tensor.matmul(
                        pq[:],
                        xT[:, cc, lc * P : (lc + 1) * P],
                        wqkvT[:, cc, 2 * J : 3 * J],
                        start=(cc == 0),
                        stop=(cc == CC - 1),
                    )
                nc.vector.tensor_copy(
                    v_aug[:, lc, :, 0:D],
                    pq[:].rearrange("p (h d) -> p h d", d=D),
                )

            # load-phase backlog: pair-0 q/k (all supers) + v blocks 0..3.
            # Borrow the idle pv/bc PSUM pools so 8 tiles are in flight while
            # the x/w chunks stream in (4 tiles starve the PE at ~2.5us/chunk).
            for ts in range(NS):
                emit_qk_tile(0, ts)
                emit_qk_tile(JC, ts)
                if ts < 2:
                    emit_v_tile(2 * ts, pool=ps_pv, tag="pv")
                    emit_v_tile(2 * ts + 1, pool=ps_bc, tag="bc")

            # ---------------- attention, super-major ----------------
            def emit_scores_exp(h, S, lb):
                bp = (h % 2) * 64
                chq = h // 2
                qTh = qkT[bp : bp + 64, chq, :]
                kTh = qkT[bp : bp + 64, JC + chq, :]
                cs, ce = S * 512, (S + 1) * 512
                l0 = lb * P
                lo = max(l0, cs)
                n = ce - lo
                sc = ps_m.tile([P, 512], F32, tag="m", name=f"sc{h}_{S}_{lb}")
                ex = exp_pool.tile([P, 512], BF16, tag="ex", name=f"ex{h}_{S}_{lb}")
                if l0 >= cs:
                    # diagonal block: score matmul + additive causal mask
                    nc.tensor.matmul(
                        sc[:, 0:P], kTh[:, l0 : l0 + P], qTh[:, lo : lo + P],
                        start=True, stop=False,
                    )
                    nc.tensor.matmul(
                        sc[:, 0:P], ident[:], maskm[:], start=False, stop=True,
                    )
                    if n > P:
                        nc.tensor.matmul(
                            sc[:, P:n], kTh[:, l0 : l0 + P], qTh[:, lo + P : ce],
                            start=True, stop=True,
                        )
                else:
                    nc.tensor.matmul(
                        sc[:, 0:n], kTh[:, l0 : l0 + P], qTh[:, lo:ce],
                        start=True, stop=True,
                    )
                nc.scalar.activation(ex[:, 0:n], sc[:, 0:n], EXPF, scale=SCALE)
                return ex, n

            def emit_pv(h, S, lb, pv, ex, n):
                nc.tensor.matmul(
                    pv[:, 512 - n :],
                    v_aug[:, lb, h, :],
                    ex[:, 0:n],
                    start=(lb == 0),
                    stop=(lb == 4 * S + 3),
                )

            def emit_norm(h, S, pv):
                """Row 64 of pv is the softmax denominator; normalize into attnT."""
                bp = (h % 2) * 64
                chq = h // 2
                cs, ce = S * 512, (S + 1) * 512
                dstage = dst_pool.tile([D, 512], F32, tag="dn", name=f"dn{h}_{S}")
                dn = dn_pool.tile([1, 512], F32, tag="dd", name=f"dd{h}_{S}")
                rec = rec_pool.tile([1, 512], F32, tag="rc", name=f"rc{h}_{S}")
                recb = recb_pool.tile([1, 512], BF16, tag="rb", name=f"rb{h}_{S}")
                bc = ps_bc.tile([D, 512], F32, tag="bc", name=f"bc{h}_{S}")
                nc.vector.tensor_copy(dn[:], pv[D : D + 1, :])
                nc.vector.reciprocal_approx_fast(rec[:], dn[:])
                nc.vector.tensor_copy(recb[:], rec[:])
                nc.vector.tensor_copy(dstage[:], pv[0:D, :])
                nc.tensor.matmul(
                    bc[:], ones_t[0:1, :], recb[0:1, :], start=True, stop=True
                )
                nc.vector.tensor_mul(
                    attnT[bp : bp + 64, chq, cs:ce], dstage[:], bc[:]
                )

            def emit_pair_super(p, S):
                # software pipeline: scores/exp(lb+1) is emitted before pv(lb)
                # so the committed PE order never parks on an exp semaphore
                for h in (2 * p, 2 * p + 1):
                    pv = ps_pv.tile([D + 1, 512], F32, tag="pv", name=f"pv{h}_{S}")
                    nlb = 4 * S + 4
                    prev = emit_scores_exp(h, S, 0)
                    for lb in range(1, nlb):
                        cur = emit_scores_exp(h, S, lb)
                        emit_pv(h, S, lb - 1, pv, *prev)
                        prev = cur
                    emit_pv(h, S, nlb - 1, pv, *prev)
                    emit_norm(h, S, pv)

            def emit_outproj_super(S, last=False):
                # on the last super the attention pools are drained: borrow
                # them so more po tiles are in flight, and split the drain
                # copies across both engines to shorten the tail
                pools = [(ps_m, "m"), (ps_pv, "pv"), (ps_bc, "bc")] if last else [
                    (ps_m, "m")
                ]
                i = 0
                for tb in range(4 * S, 4 * S + 4):
                    for oc in range(2):
                        pool, tg = pools[i % len(pools)]
                        po = pool.tile([P, 512], F32, tag=tg, name=f"o{tb}_{oc}")
                        for jc in range(JC):
                            nc.tensor.matmul(
                                po[:],
                                attnT[:, jc, tb * P : (tb + 1) * P],
                                woutT[:, jc, oc * 512 : (oc + 1) * 512],
                                start=(jc == 0),
                                stop=(jc == JC - 1),
                            )
                        ot = out_pool.tile([P, 512], F32, tag="ot", name=f"ot{tb}_{oc}")
                        if last and i % 2 == 1:
                            nc.scalar.copy(ot[:], po[:])
                        else:
                            nc.vector.tensor_copy(ot[:], po[:])
                        nc.sync.dma_start(
                            y_d[tb][:, oc * 512 : (oc + 1) * 512], ot[:]
                        )
                        i += 1

            for S in range(NS):
                for p in range(JC):
                    if S == 0 and p > 0:
                        # this pair's super-0 q/k tiles (needed right now)
                        emit_qk_tile(p, 0)
                        emit_qk_tile(JC + p, 0)
                    if S + 1 < NS:
                        # next super's q/k tiles for this head pair (filler)
                        emit_qk_tile(p, S + 1)
                        emit_qk_tile(JC + p, S + 1)
                    emit_pair_super(p, S)
                if S > 0:
                    emit_outproj_super(S - 1)
                if S + 1 < NS:
                    # v blocks needed from super S+1 (lb up to 4(S+1)+3)
                    for lc in range(4 * S + 4, 4 * S + 8):
                        emit_v_tile(lc)
            emit_outproj_super(NS - 1, last=True)

    nc.compile()
    return nc


_CACHE = {}

# Set by test harnesses to capture a profile; harmless defaults for grading.
TRACE = False
LAST_RESULT = None


def get_program(T=2048):
    if T not in _CACHE:
        _CACHE[T] = build_program(T)
    return _CACHE[T]


def make_in_map(x_b, w_qkv, w_out, hg, T=2048):
    """Host-side shard prep for one core: batch slice x_b [T, C], head group hg."""
    xT = np.ascontiguousarray(x_b.T).astype(NPBF16).reshape(CC, P, T)
    W = np.concatenate(
        [
            w_qkv[hg * J : (hg + 1) * J],
            w_qkv[C + hg * J : C + (hg + 1) * J],
            w_qkv[2 * C + hg * J : 2 * C + (hg + 1) * J],
        ],
        axis=0,
    )  # [3J, C]
    wqkvT = np.ascontiguousarray(W.T).astype(NPBF16).reshape(CC, P, 3 * J)
    Wo = w_out[:, hg * J : (hg + 1) * J]  # [C, J]
    woutT = np.ascontiguousarray(Wo.T).astype(NPBF16).reshape(JC, P, C)
    ident = np.eye(P, dtype=np.float32).astype(NPBF16)
    # additive causal mask for the diagonal 128-block, [l_local, i_local]:
    # invalid where i_local < l_local
    maskm = np.where(
        np.arange(P)[None, :] < np.arange(P)[:, None], -1e6, 0.0
    ).astype(NPBF16)
    return {"xT": xT, "wqkvT": wqkvT, "woutT": woutT, "ident": ident, "maskm": maskm}


def kernel(x, w_qkv, w_out, b_out):
    x = np.asarray(x, dtype=np.float32)
    w_qkv = np.asarray(w_qkv, dtype=np.float32)
    w_out = np.asarray(w_out, dtype=np.float32)
    b_out = np.asarray(b_out, dtype=np.float32)
    B, T, Cx = x.shape
    assert Cx == C

    nc = get_program(T)
    in_maps = [
        make_in_map(x[core // 2], w_qkv, w_out, core % 2, T) for core in range(8)
    ]
    res = run_bass_kernel_spmd(nc, in_maps, core_ids=list(range(8)), trace=TRACE)
    global LAST_RESULT
    LAST_RESULT = res
    outs = [r["y"].reshape(T, C).astype(np.float32) for r in res.results]
    y = np.stack([outs[2 * b] + outs[2 * b + 1] for b in range(B)])
    return (y + b_out[None, None, :]).astype(np.float32)
